# revision 1
# baseline (speedup 1.0000x reference)
"""DGCNN (4x EdgeConv + FC head) Bass kernel for 8 trn2 NeuronCores.

Sharding: cloud b -> cores {2b, 2b+1}; each core owns 1024 query points of its
cloud (q0 = (partition_id % 2) * 1024). Full cloud features are exchanged
within each pair via AllGather after layers 1-3.

Self-contained: hardcodes B=4, P=2048, K=20 and the model dims.
"""
import numpy as np

import concourse.bass as bass
import concourse.bacc as bacc
import concourse.mybir as mybir
import concourse.tile as tile
from concourse.bass_utils import run_bass_kernel_spmd
from concourse.masks import make_identity

B, P, K = 4, 2048, 20
NQ = 1024            # queries owned per core
N_CORES = 8
NEG = -3.0e38
LDIMS = [(3, 64, 64), (64, 128, 128), (128, 256, 256), (256, 512, 512)]
F32 = mybir.dt.float32
F32R = mybir.dt.float32r
AF = mybir.ActivationFunctionType
ALU = mybir.AluOpType
CCHUNK = 2           # neighbors per dma_gather (num_idxs = 128*CCHUNK <= 512)

_CACHED = {}


def cdiv(a, b):
    return (a + b - 1) // b


def _build():
    nc = bacc.Bacc("TRN2", target_bir_lowering=False, debug=False,
                   num_devices=N_CORES, num_swdge_queues=4)

    xT_in = nc.declare_dram_parameter("xT", [3, P], F32, isOutput=False)
    nsq_in = nc.declare_dram_parameter("nsq", [1, P], F32, isOutput=False)
    wparams = {}
    for li, (D, DH, DO) in enumerate(LDIMS, start=1):
        wparams[f"wdiff{li}"] = nc.declare_dram_parameter(f"wdiff{li}", [D, DH], F32, isOutput=False)
        wparams[f"wbot{li}"] = nc.declare_dram_parameter(f"wbot{li}", [D, DH], F32, isOutput=False)
        wparams[f"ba{li}"] = nc.declare_dram_parameter(f"ba{li}", [1, DH], F32, isOutput=False)
        wparams[f"wb{li}"] = nc.declare_dram_parameter(f"wb{li}", [DH, DO], F32, isOutput=False)
        wparams[f"bb{li}"] = nc.declare_dram_parameter(f"bb{li}", [DO, 1], F32, isOutput=False)
    wparams["fw1"] = nc.declare_dram_parameter("fw1", [960, 512], F32, isOutput=False)
    wparams["fb1"] = nc.declare_dram_parameter("fb1", [1, 512], F32, isOutput=False)
    wparams["fw2"] = nc.declare_dram_parameter("fw2", [512, 256], F32, isOutput=False)
    wparams["fb2"] = nc.declare_dram_parameter("fb2", [1, 256], F32, isOutput=False)
    wparams["fw3"] = nc.declare_dram_parameter("fw3", [256, 1], F32, isOutput=False)
    wparams["fb3"] = nc.declare_dram_parameter("fb3", [1, 1], F32, isOutput=False)
    y_out = nc.declare_dram_parameter("y", [1, NQ], F32, isOutput=True)

    groups = [[2 * b, 2 * b + 1] for b in range(N_CORES // 2)]

    with tile.TileContext(nc) as tc:
        with tc.tile_pool(name="const", bufs=1) as cpool, \
             tc.tile_pool(name="xping", bufs=1) as xping, \
             tc.tile_pool(name="xpong", bufs=1) as xpong, \
             tc.tile_pool(name="dram", bufs=1, space="DRAM") as dram:

            ident = cpool.tile([128, 128], F32)
            make_identity(nc, ident[:])
            onesr = cpool.tile([1, 1024], F32)
            nc.vector.memset(onesr[:], 1.0)
            onesr_r = cpool.tile([1, 1024], F32R)
            nc.vector.tensor_copy(onesr_r[:], onesr[:])
            onescol = cpool.tile([128, 1], F32)
            nc.vector.memset(onescol[:], 1.0)

            def load_round(pool, name, shape, row_chunks=None):
                """DRAM fp32 -> SBUF f32r tiles split at given row boundaries."""
                src = wparams[name]
                if row_chunks is None:
                    row_chunks = []
                    r = shape[0]
                    while r > 0:
                        row_chunks.append(min(128, r))
                        r -= 128
                tiles, c0 = [], 0
                for rows in row_chunks:
                    t32 = pool.tile([rows, shape[1]], F32, name=f"{name}_f{c0}",
                                    tag="wstage", bufs=2)
                    nc.sync.dma_start(t32[:], src[c0:c0 + rows, :])
                    tr = pool.tile([rows, shape[1]], F32R, name=f"{name}_r{c0}",
                                   tag=f"{name}_r{c0}")
                    nc.vector.tensor_copy(tr[:], t32[:])
                    tiles.append(tr)
                    c0 += rows
                return tiles

            q0 = nc.vector.partition_id()
            q0 = nc.vector.scalar_reg_alu(ALU.mod, q0, 2)
            q0 = nc.vector.scalar_reg_alu(ALU.mult, q0, NQ)

            nsq_dram = [nsq_in] + [dram.tile([1, P], F32, name=f"nsqd{li}")
                                   for li in (2, 3, 4)]
            xown_dram = [dram.tile([do, NQ], F32, name=f"xown{li}")
                         for li, (_, _, do) in enumerate(LDIMS, start=1)]
            ag_out = [dram.tile([2 * do, NQ], F32, name=f"agout{li}")
                      for li, (_, _, do) in enumerate(LDIMS[:3], start=1)]

            xT_tiles = None

            for li, (D, DH, DO) in enumerate(LDIMS, start=1):
                NDC = cdiv(D, 128)
                NHC = cdiv(DH, 128)
                NMC = cdiv(DO, 128)
                v_dram = dram.tile([P, DH], F32, name=f"vdram{li}")
                idx_dram = dram.tile([128, 8 * K], mybir.dt.int16, name=f"idxd{li}")

                with tc.tile_pool(name=f"l{li}", bufs=1) as lp, \
                     tc.tile_pool(name=f"l{li}w", bufs=2) as wp, \
                     tc.tile_pool(name=f"l{li}wt", bufs=1) as lw:

                    Wd = load_round(lw, f"wdiff{li}", (D, DH))
                    Wb = load_round(lw, f"wbot{li}", (D, DH))
                    Wba = load_round(lw, f"ba{li}", (1, DH))
                    Wwb = load_round(lw, f"wb{li}", (DH, DO))
                    Wbb = []
                    for m in range(NMC):
                        mrows = min(128, DO - m * 128)
                        bt = lw.tile([mrows, 1], F32, name=f"bb{li}_t{m}",
                                     tag=f"bb{li}_t{m}")
                        nc.sync.dma_start(bt[:], wparams[f"bb{li}"][m * 128:m * 128 + mrows, :])
                        Wbb.append(bt)

                    if li == 1:
                        t = lp.tile([3, P], F32, name="xT1")
                        nc.sync.dma_start(t[:], xT_in[:, :])
                        xT_tiles = [t]

                    xTr_tiles = []
                    for ci, xt in enumerate(xT_tiles):
                        tr = lp.tile([xt.shape[0], P], F32R, name=f"xTr{ci}")
                        nc.vector.tensor_copy(tr[:], xt[:])
                        xTr_tiles.append(tr)

                    QT, QTr = [], []
                    for ci, xt in enumerate(xT_tiles):
                        qt = lp.tile([xt.shape[0], NQ], F32, name=f"QT{ci}")
                        nc.vector.tensor_copy(qt[:], xt[:, bass.ds(q0, NQ)])
                        QT.append(qt)
                        qtr = lp.tile([xt.shape[0], NQ], F32R, name=f"QTr{ci}")
                        nc.vector.tensor_copy(qtr[:], xTr_tiles[ci][:, bass.ds(q0, NQ)])
                        QTr.append(qtr)

                    nsqb = lp.tile([128, P], F32, name="nsqb")
                    nc.sync.dma_start(nsqb[:],
                                      nsq_dram[li - 1][0:1, :].to_broadcast([128, P]))

                    # ---- phase 1: v, u, dist+topk (psum pool) ----
                    with tc.tile_pool(name=f"l{li}p1", bufs=2, space="PSUM") as pp1:
                        for pt in range(P // 128):
                            vps = pp1.tile([128, DH], F32, name="vps", tag="vps")
                            for ci in range(NDC):
                                nc.tensor.matmul(
                                    vps[:], xTr_tiles[ci][:, pt * 128:(pt + 1) * 128],
                                    Wb[ci][:], start=(ci == 0), stop=(ci == NDC - 1))
                            vrow = wp.tile([128, DH], F32, name="vrow", tag="vrow")
                            nc.scalar.activation(vrow[:], vps[:], AF.Copy)
                            nc.sync.dma_start(v_dram[pt * 128:(pt + 1) * 128, :], vrow[:])

                        urow_tiles = []
                        for pt in range(NQ // 128):
                            ups = pp1.tile([128, DH], F32, name="ups", tag="vps")
                            for ci in range(NDC):
                                nc.tensor.matmul(
                                    ups[:], QTr[ci][:, pt * 128:(pt + 1) * 128],
                                    Wd[ci][:], start=(ci == 0), stop=False)
                            nc.tensor.matmul(ups[:],
                                             onesr_r[:, pt * 128:(pt + 1) * 128],
                                             Wba[0][:], start=False, stop=True)
                            ur = lp.tile([128, DH], F32, name=f"urow{pt}")
                            nc.scalar.activation(ur[:], ups[:], AF.Copy)
                            urow_tiles.append(ur)

                        idx16 = lp.tile([128, 8 * K], mybir.dt.int16, name="idx16")
                        score = lp.tile([128, P], F32, name="score")
                        maxv = lp.tile([128, 24], F32, name="maxv")
                        idxs = lp.tile([128, 24], mybir.dt.uint32, name="idxs")
                        for t in range(NQ // 128):
                            for hb in range(2):
                                dps = pp1.tile([128, 1024], F32, name="dps", tag="dps")
                                for nb in range(2):
                                    sl = slice(hb * 1024 + nb * 512,
                                               hb * 1024 + (nb + 1) * 512)
                                    psl = slice(nb * 512, (nb + 1) * 512)
                                    for ci in range(NDC):
                                        nc.tensor.matmul(
                                            dps[:, psl],
                                            QT[ci][:, t * 128:(t + 1) * 128],
                                            xT_tiles[ci][:, sl],
                                            start=(ci == 0), stop=(ci == NDC - 1))
                                nc.vector.tensor_tensor(
                                    score[:, hb * 1024:(hb + 1) * 1024], dps[:],
                                    nsqb[:, hb * 1024:(hb + 1) * 1024], op=ALU.add)
                            for r in range(3):
                                nc.vector.max(maxv[:, 8 * r:8 * r + 8], score[:])
                                nc.vector.max_index(idxs[:, 8 * r:8 * r + 8],
                                                    maxv[:, 8 * r:8 * r + 8], score[:])
                                if r < 2:
                                    nc.vector.match_replace(
                                        score[:], maxv[:, 8 * r:8 * r + 8], score[:], NEG)
                            nc.vector.tensor_copy(idx16[:, t * K:(t + 1) * K],
                                                  idxs[:, :K])

                    # ---- wrap indices ----
                    nc.sync.dma_start(idx_dram[:, :], idx16[:])
                    wrapped = lp.tile([128, 8 * K * 8], mybir.dt.int16, name="wrapped")
                    vsrc = idx_dram[:, :].rearrange("(r q) tc -> q tc r", r=8, q=16)
                    for bb_ in range(8):
                        nc.sync.dma_start(
                            wrapped[bb_ * 16:(bb_ + 1) * 16, :].rearrange(
                                "q (tc r) -> q tc r", r=8),
                            vsrc)

                    # ---- phase 2: edge MLP + max over k ----
                    gacc = [lp.tile([128, 512], F32, name=f"acc{g}_{m}")
                            for g in range(2) for m in range(NMC)]
                    with tc.tile_pool(name=f"l{li}p2", bufs=8, space="PSUM") as pp2:
                        for g in range(2):
                            tiles4 = [g * 4 + i for i in range(4)]
                            for c0 in range(0, K, CCHUNK):
                                vk = {}
                                for tt in tiles4:
                                    vkt = wp.tile([128, CCHUNK, DH], F32,
                                                  name=f"vk{tt % 4}", tag=f"vk{tt % 4}")
                                    nc.gpsimd.dma_gather(
                                        out_ap=vkt[:], in_ap=v_dram[:, :],
                                        idxs_ap=wrapped[:, tt * K * 8 + c0 * 8:
                                                        tt * K * 8 + (c0 + CCHUNK) * 8],
                                        num_idxs=128 * CCHUNK,
                                        num_idxs_reg=128 * CCHUNK,
                                        elem_size=DH, queue_num=tt % 4)
                                    vk[tt] = vkt
                                for kk in range(CCHUNK):
                                    h1 = []
                                    for h in range(NHC):
                                        rows = min(128, DH - h * 128)
                                        hps = pp2.tile([rows, 512], F32,
                                                       name="hps", tag="bank")
                                        for j, tt in enumerate(tiles4):
                                            nc.tensor.matmul(
                                                hps[:, j * 128:(j + 1) * 128],
                                                vk[tt][:, kk, h * 128:h * 128 + rows],
                                                ident[:], is_transpose=True,
                                                start=(j == 0), stop=False)
                                            nc.tensor.matmul(
                                                hps[:, j * 128:(j + 1) * 128],
                                                urow_tiles[tt][:, h * 128:h * 128 + rows],
                                                ident[:], is_transpose=True,
                                                start=False, stop=(j == 3))
                                        h1t = wp.tile([rows, 512], F32R,
                                                      name=f"h1t{h}", tag=f"h1t{h}")
                                        nc.scalar.activation(h1t[:], hps[:], AF.Relu)
                                        h1.append(h1t)
                                    for m in range(NMC):
                                        mrows = min(128, DO - m * 128)
                                        h2ps = pp2.tile([mrows, 512], F32,
                                                        name="h2ps", tag="bank")
                                        for h in range(NHC):
                                            nc.tensor.matmul(
                                                h2ps[:],
                                                Wwb[h][:, m * 128:m * 128 + mrows],
                                                h1[h][:],
                                                start=(h == 0), stop=(h == NHC - 1))
                                        am = gacc[g * NMC + m]
                                        if c0 == 0 and kk == 0:
                                            nc.vector.tensor_copy(am[:mrows, :], h2ps[:])
                                        else:
                                            nc.vector.tensor_tensor(
                                                am[:mrows, :], h2ps[:], am[:mrows, :],
                                                op=ALU.max)

                    # ---- relu(acc + bb) -> own xT block -> DRAM ----
                    for m in range(NMC):
                        mrows = min(128, DO - m * 128)
                        for g in range(2):
                            xo = wp.tile([128, 512], F32, name="xo", tag="xo")
                            nc.scalar.activation(
                                xo[:mrows, :], gacc[g * NMC + m][:mrows, :],
                                AF.Relu, bias=Wbb[m][:mrows, :])
                            nc.sync.dma_start(
                                xown_dram[li - 1][m * 128:m * 128 + mrows,
                                                  g * 512:(g + 1) * 512],
                                xo[:mrows, :])

                    if li < 4:
                        nc.gpsimd.collective_compute(
                            "AllGather", ALU.bypass, replica_groups=groups,
                            ins=[xown_dram[li - 1].opt()],
                            outs=[ag_out[li - 1].opt()])

                if li < 4:
                    xp = xping if li % 2 == 1 else xpong
                    with tc.tile_pool(name=f"x{li}ps", bufs=1, space="PSUM") as xpp, \
                         tc.tile_pool(name=f"x{li}tmp", bufs=1) as xtmp:
                        xT_tiles = []
                        for m in range(NMC):
                            mrows = min(128, DO - m * 128)
                            xt = xp.tile([mrows, P], F32, name=f"xTn{li}_{m}",
                                         tag=f"xTn{li}_{m}")
                            nc.sync.dma_start(
                                xt[:, 0:NQ],
                                ag_out[li - 1][m * 128:m * 128 + mrows, :])
                            nc.sync.dma_start(
                                xt[:, NQ:P],
                                ag_out[li - 1][DO + m * 128:DO + m * 128 + mrows, :])
                            xT_tiles.append(xt)
                        sq = xtmp.tile([128, P], F32, name="sqtmp")
                        sps = xpp.tile([1, P], F32, name="sps", space="PSUM")
                        for m in range(NMC):
                            mrows = min(128, DO - m * 128)
                            nc.vector.tensor_tensor(sq[:mrows, :], xT_tiles[m][:],
                                                    xT_tiles[m][:], op=ALU.mult)
                            for nb in range(P // 512):
                                nc.tensor.matmul(
                                    sps[:, nb * 512:(nb + 1) * 512],
                                    onescol[:mrows, :],
                                    sq[:mrows, nb * 512:(nb + 1) * 512],
                                    start=(m == 0), stop=(m == NMC - 1))
                        nsqrow = xtmp.tile([1, P], F32, name="nsqrow")
                        nc.scalar.activation(nsqrow[:], sps[:], AF.Copy, scale=-0.5)
                        nc.sync.dma_start(nsq_dram[li][0:1, :], nsqrow[:])

            # ---------------- FC head ----------------
            with tc.tile_pool(name="fc", bufs=1) as fp, \
                 tc.tile_pool(name="fcw", bufs=1) as fw, \
                 tc.tile_pool(name="fcps", bufs=2, space="PSUM") as fpp:
                feat_chunks = [64, 128, 128, 128, 128, 128, 128, 128]
                Wf1 = load_round(fw, "fw1", (960, 512), row_chunks=feat_chunks)
                Wfb1 = load_round(fw, "fb1", (1, 512))
                Wf2 = load_round(fw, "fw2", (512, 256))
                Wfb2 = load_round(fw, "fb2", (1, 256))
                Wf3 = load_round(fw, "fw3", (256, 1))
                Wfb3 = load_round(fw, "fb3", (1, 1))

                feat_r = []
                for li, (_, _, do) in enumerate(LDIMS, start=1):
                    for m in range(cdiv(do, 128)):
                        mrows = min(128, do - m * 128)
                        f32t = fp.tile([mrows, NQ], F32, name=f"ff{li}_{m}")
                        nc.sync.dma_start(
                            f32t[:], xown_dram[li - 1][m * 128:m * 128 + mrows, :])
                        frt = fp.tile([mrows, NQ], F32R, name=f"fr{li}_{m}")
                        nc.vector.tensor_copy(frt[:], f32t[:])
                        feat_r.append(frt)

                def fc_layer(rhs_chunks, wtiles, btile, nout, act):
                    outs = []
                    for m in range(cdiv(nout, 128)):
                        mrows = min(128, nout - m * 128)
                        ot = fp.tile([mrows, NQ], F32R if act == AF.Relu else F32,
                                     name=f"fco{nout}_{m}")
                        for g in range(2):
                            ps = fpp.tile([mrows, 512], F32, name="fps", tag="fcps",
                                          space="PSUM")
                            for ci, rc in enumerate(rhs_chunks):
                                nc.tensor.matmul(
                                    ps[:], wtiles[ci][:, m * 128:m * 128 + mrows],
                                    rc[:, g * 512:(g + 1) * 512],
                                    start=(ci == 0), stop=False)
                            nc.tensor.matmul(
                                ps[:], btile[0][:, m * 128:m * 128 + mrows],
                                onesr_r[:, g * 512:(g + 1) * 512],
                                start=False, stop=True)
                            nc.scalar.activation(ot[:, g * 512:(g + 1) * 512],
                                                 ps[:], act)
                        outs.append(ot)
                    return outs

                h1fc = fc_layer(feat_r, Wf1, Wfb1, 512, AF.Relu)
                h2fc = fc_layer(h1fc, Wf2, Wfb2, 256, AF.Relu)
                yt = fc_layer(h2fc, Wf3, Wfb3, 1, AF.Sigmoid)
                nc.sync.dma_start(y_out[:, :], yt[0][:])

    nc.compile()
    return nc


def kernel(**inputs):
    x = np.asarray(inputs["x"], np.float32)          # [8192, 3]
    if "nc" not in _CACHED:
        _CACHED["nc"] = _build()
    nc = _CACHED["nc"]

    base = {}
    for li in range(1, 5):
        wa = np.asarray(inputs[f"w{li}a"], np.float32)
        D = wa.shape[0] // 2
        base[f"wdiff{li}"] = np.ascontiguousarray(wa[:D] - wa[D:])
        base[f"wbot{li}"] = np.ascontiguousarray(wa[D:])
        base[f"ba{li}"] = np.asarray(inputs[f"b{li}a"], np.float32)[None, :]
        base[f"wb{li}"] = np.asarray(inputs[f"w{li}b"], np.float32)
        base[f"bb{li}"] = np.asarray(inputs[f"b{li}b"], np.float32)[:, None]
    for nm in ("fw1", "fw2", "fw3"):
        base[nm] = np.asarray(inputs[nm], np.float32)
    for nm in ("fb1", "fb2", "fb3"):
        base[nm] = np.asarray(inputs[nm], np.float32)[None, :]

    in_maps = []
    for c in range(N_CORES):
        cloud = c // 2
        xc = x[cloud * P:(cloud + 1) * P]
        m = dict(base)
        m["xT"] = np.ascontiguousarray(xc.T)
        m["nsq"] = (-0.5 * (xc * xc).sum(1))[None, :].astype(np.float32)
        in_maps.append(m)

    res = run_bass_kernel_spmd(nc, in_maps, core_ids=list(range(N_CORES)))
    out = np.empty((B * P, 1), np.float32)
    for c in range(N_CORES):
        cloud, half = c // 2, c % 2
        out[cloud * P + half * NQ: cloud * P + (half + 1) * NQ, 0] = \
            res.results[c]["y"][0]
    return out



# revision 38
# speedup vs baseline: 1.0481x; 1.0481x over previous
"""DGCNN (4x EdgeConv + FC head) Bass kernel for 8 trn2 NeuronCores — v2.

Sharding: cloud b -> cores {2b, 2b+1}; each core owns NQ=1024 query points
(q0 = (partition_id % 2) * 1024). Full-cloud features exchanged within each
pair via bf16 AllGather after layers 1-3.

v2 design vs v1:
- dist matmul: rank-1 fold of -0.5|x|^2 (and bias) into the matmul; bf16
  features for layers 2-4 (f32r for layer 1 coords).
- topk: index bits embedded into score mantissa (score&~0x7FF)|iota, then
  chunked max8 (8x256) + top-24-of-64 rounds; indices recovered by masking.
  No MaxIndex scans at all.
- neighbor features v stored bf16 in DRAM; gathered with transpose-mode
  dma_gather directly into [DH, edge] layout (no per-edge PE transposes).
- h1 = relu(uT + vT) on DVE (bf16 2x/4x); h2 = wb.T @ h1 (bf16 matmuls);
- per-k relu+bias on ACT (psum->sbuf bf16, unwrapping the gather slot
  permutation via a strided write AP); max over k on DVE in bf16.
- FC head runs from SBUF-resident features, biases applied as ACT bias.
"""
import numpy as np

import concourse.bass as bass
import concourse.bacc as bacc
import concourse.mybir as mybir
import concourse.tile as tile
from concourse.bass_utils import run_bass_kernel_spmd
from concourse.masks import make_identity

B, P, K = 4, 2048, 20
NQ = 1024
N_CORES = 8
NEG = -3.0e38
LDIMS = [(3, 64, 64), (64, 128, 128), (128, 256, 256), (256, 512, 512)]
F32 = mybir.dt.float32
F32R = mybir.dt.float32r
BF = mybir.dt.bfloat16
U32 = mybir.dt.uint32
I16 = mybir.dt.int16
AF = mybir.ActivationFunctionType
ALU = mybir.AluOpType
NIDX = 512           # indices per gather call (per-queue SWDGE ring = 1024 descs)
NCH = 8              # score chunks for phase-A max8
CHW = P // NCH       # 256

_CACHED = {}


def cdiv(a, b):
    return (a + b - 1) // b


def _build(nidx=NIDX):
    nc = bacc.Bacc("TRN2", target_bir_lowering=False, debug=False,
                   num_devices=N_CORES, num_swdge_queues=4)

    xT_in = nc.declare_dram_parameter("xT", [3, P], F32, isOutput=False)
    nsq_in = nc.declare_dram_parameter("nsq", [1, P], F32, isOutput=False)
    iota_in = nc.declare_dram_parameter("iota", [1, P], F32, isOutput=False)
    wparams = {}
    for li, (D, DH, DO) in enumerate(LDIMS, start=1):
        wparams[f"wdiff{li}"] = nc.declare_dram_parameter(f"wdiff{li}", [D, DH], F32, isOutput=False)
        wparams[f"wbot{li}"] = nc.declare_dram_parameter(f"wbot{li}", [D, DH], F32, isOutput=False)
        wparams[f"ba{li}"] = nc.declare_dram_parameter(f"ba{li}", [1, DH], F32, isOutput=False)
        wparams[f"wb{li}"] = nc.declare_dram_parameter(f"wb{li}", [DH, DO], F32, isOutput=False)
        wparams[f"bb{li}"] = nc.declare_dram_parameter(f"bb{li}", [DO, 1], F32, isOutput=False)
    wparams["fw1"] = nc.declare_dram_parameter("fw1", [960, 512], F32, isOutput=False)
    wparams["fb1"] = nc.declare_dram_parameter("fb1", [512, 1], F32, isOutput=False)
    wparams["fw2"] = nc.declare_dram_parameter("fw2", [512, 256], F32, isOutput=False)
    wparams["fb2"] = nc.declare_dram_parameter("fb2", [256, 1], F32, isOutput=False)
    wparams["fw3"] = nc.declare_dram_parameter("fw3", [256, 1], F32, isOutput=False)
    wparams["fb3"] = nc.declare_dram_parameter("fb3", [1, 1], F32, isOutput=False)
    y_out = nc.declare_dram_parameter("y", [1, NQ], F32, isOutput=True)

    groups = [[2 * b, 2 * b + 1] for b in range(N_CORES // 2)]
    NC = nidx // 16      # idx columns per gather call in wrapped layout
    NGC = NQ // nidx     # gather calls per k (1 when nidx=1024)

    with tile.TileContext(nc) as tc:
        with tc.tile_pool(name="const", bufs=1) as cpool, \
             tc.tile_pool(name="feat", bufs=1) as fpool, \
             tc.tile_pool(name="dram", bufs=1, space="DRAM") as dram:

            ident = cpool.tile([128, 128], F32)
            make_identity(nc, ident[:])
            ones_bf = cpool.tile([1, P], BF)
            nc.vector.memset(ones_bf[:], 1.0)
            ones_f = cpool.tile([1, NQ], F32)
            nc.vector.memset(ones_f[:], 1.0)
            ones_r = cpool.tile([1, NQ], F32R)
            nc.vector.tensor_copy(ones_r[:], ones_f[:])
            onescol_bf = cpool.tile([128, 1], BF)
            nc.vector.memset(onescol_bf[:], 1.0)
            iota_bc = cpool.tile([128, P], F32)
            nc.sync.dma_start(iota_bc[:], iota_in[0:1, :].to_broadcast([128, P]))
            mask_hi = cpool.tile([128, 1], U32)
            nc.vector.memset(mask_hi[:], 0xFFFFF800)
            mask_lo = cpool.tile([128, 1], U32)
            nc.vector.memset(mask_lo[:], 0x7FF)

            q0 = nc.vector.partition_id()
            q0 = nc.vector.scalar_reg_alu(ALU.mod, q0, 2)
            q0 = nc.vector.scalar_reg_alu(ALU.mult, q0, NQ)

            def load_bf(pool, name, shape, row_chunks=None, tagpfx=None):
                """DRAM fp32 -> SBUF bf16 tiles chunked along rows."""
                src = wparams[name]
                if row_chunks is None:
                    row_chunks = []
                    r = shape[0]
                    while r > 0:
                        row_chunks.append(min(128, r))
                        r -= 128
                tiles, c0 = [], 0
                tagpfx = tagpfx or name
                for rows in row_chunks:
                    t32 = pool.tile([rows, shape[1]], F32, name=f"{name}_f{c0}",
                                    tag="wstage", bufs=2)
                    nc.sync.dma_start(t32[:], src[c0:c0 + rows, :])
                    tb = pool.tile([rows, shape[1]], BF, name=f"{name}_b{c0}",
                                   tag=f"{tagpfx}_b{c0}")
                    nc.vector.tensor_copy(tb[:], t32[:])
                    tiles.append(tb)
                    c0 += rows
                return tiles

            # ---- layer-1 input staging (f32r coords + nsq) ----
            Xp1 = fpool.tile([4, P], F32R, name="Xp1")
            Qp1 = fpool.tile([4, NQ], F32R, name="Qp1")
            with tc.tile_pool(name="stage1", bufs=1) as st1:
                # assemble [x; nsq] and [x_q; ones] in fp32 via DMA (which has
                # no partition-alignment limits), then round to f32r in one
                # aligned DVE copy each.
                x4f = st1.tile([4, P], F32, name="x4f")
                nc.sync.dma_start(x4f[0:3, :], xT_in[:, :])
                nc.sync.dma_start(x4f[3:4, :], nsq_in[0:1, :])
                nc.vector.tensor_copy(Xp1[:], x4f[:])
                q4f = st1.tile([4, NQ], F32, name="q4f")
                nc.vector.tensor_copy(q4f[0:3, :], x4f[0:3, bass.ds(q0, NQ)])
                nc.sync.dma_start(q4f[3:4, :], ones_f[:])
                nc.vector.tensor_copy(Qp1[:], q4f[:])

            # persistent per-layer outputs (own half, natural order, bf16)
            acc_tiles = {}      # li -> list of [mrows, NQ] bf16 tiles
            xown_dram = [dram.tile([do, NQ], BF, name=f"xown{li}")
                         for li, (_, _, do) in enumerate(LDIMS, start=1)]
            ag_out = [dram.tile([2 * do, NQ], BF, name=f"agout{li}")
                      for li, (_, _, do) in enumerate(LDIMS[:3], start=1)]

            # feature chunks for the NEXT layer (full cloud, bf16) and nsq row
            feat_chunks = None   # list of [rows<=128, P] bf16 tiles
            nsq_row = None       # [1, P] bf16

            for li, (D, DH, DO) in enumerate(LDIMS, start=1):
                NDC = cdiv(D, 128)     # feature chunks (layers 2-4)
                NHC = cdiv(DH, 128)
                NMC = cdiv(DO, 128)
                DHP = max(DH, 128)     # padded row length of v in DRAM
                v_dram = dram.tile([P, DHP], BF, name=f"vdram{li}")
                idxT_dram = dram.tile([K, NQ], I16, name=f"idxTd{li}")

                with tc.tile_pool(name=f"l{li}w", bufs=1) as lw, \
                     tc.tile_pool(name=f"l{li}f", bufs=1) as lp, \
                     tc.tile_pool(name=f"l{li}s", bufs=2) as sp:

                    if li == 1:
                        Wd = [cpool.tile([3, DH], F32R, name="wd1r")]
                        Wb_ = [cpool.tile([3, DH], F32R, name="wb1r")]
                        Ba = cpool.tile([1, DH], F32R, name="ba1r")
                        for nm, dst in (("wdiff1", Wd[0]), ("wbot1", Wb_[0]),
                                        ("ba1", Ba)):
                            t32 = lw.tile(list(dst.shape), F32, name=f"{nm}_f",
                                          tag="wstage1", bufs=2)
                            nc.sync.dma_start(t32[:], wparams[nm][:, :])
                            nc.vector.tensor_copy(dst[:], t32[:])
                    else:
                        Wd = load_bf(lw, f"wdiff{li}", (D, DH))
                        Wb_ = load_bf(lw, f"wbot{li}", (D, DH))
                        Ba = load_bf(lw, f"ba{li}", (1, DH))[0]
                    Wwb = load_bf(lw, f"wb{li}", (DH, DO))
                    bbt = []
                    for m in range(NMC):
                        mrows = min(128, DO - m * 128)
                        bt = lw.tile([mrows, 1], F32, name=f"bb{li}t{m}",
                                     tag=f"bb{li}t{m}")
                        nc.sync.dma_start(
                            bt[:], wparams[f"bb{li}"][m * 128:m * 128 + mrows, :])
                        bbt.append(bt)

                    # query columns: own-half features are the previous
                    # layer's acc tiles (pre-collective, so uT/dist-lhsT
                    # work can overlap the AllGather)
                    if li == 1:
                        qfeat = [Qp1[0:3, :]]
                    else:
                        qfeat = [a[:] for a in acc_tiles[li - 1]]
                    # slot-space view per gather group of `nidx` queries:
                    # slot j = s*16+q -> query g*nidx + q*NC + s
                    qperm = [ap.rearrange("d (g q s) -> d g s q",
                                          g=NGC, q=16, s=NC)
                             for ap in qfeat]
                    # idx scratch
                    idxT_sb = lp.tile([K, NQ], I16, name="idxT")

                    # ---------- phase 1: dist + topk ----------
                    with tc.tile_pool(name=f"l{li}p1", bufs=2, space="PSUM") as pp1:
                        for t in range(NQ // 128):
                            score = sp.tile([128, P], F32, name="score", tag="score")
                            for hb in range(2):
                                dps = pp1.tile([128, 1024], F32, name="dps", tag="dps")
                                for nb in range(2):
                                    psl = slice(nb * 512, (nb + 1) * 512)
                                    sl = slice(hb * 1024 + nb * 512,
                                               hb * 1024 + (nb + 1) * 512)
                                    if li == 1:
                                        nc.tensor.matmul(
                                            dps[:, psl],
                                            Qp1[:, t * 128:(t + 1) * 128],
                                            Xp1[:, sl], start=True, stop=True)
                                    else:
                                        for ci in range(NDC):
                                            nc.tensor.matmul(
                                                dps[:, psl],
                                                qfeat[ci][:, t * 128:(t + 1) * 128],
                                                feat_chunks[ci][:, sl],
                                                start=(ci == 0), stop=False)
                                        nc.tensor.matmul(
                                            dps[:, psl],
                                            ones_bf[:, t * 128:(t + 1) * 128],
                                            nsq_row[:, sl],
                                            start=False, stop=True)
                                nc.scalar.activation(
                                    score[:, hb * 1024:(hb + 1) * 1024], dps[:],
                                    AF.Copy)
                            # embed candidate index into low mantissa bits
                            nc.vector.scalar_tensor_tensor(
                                score[:].bitcast(U32), score[:].bitcast(U32),
                                mask_hi[:], iota_bc[:].bitcast(U32),
                                ALU.bitwise_and, ALU.bitwise_or)
                            cand = sp.tile([128, 8 * NCH], F32, name="cand",
                                           tag="cand")
                            for c in range(NCH):
                                nc.vector.max(cand[:, 8 * c:8 * c + 8],
                                              score[:, CHW * c:CHW * (c + 1)])
                            win = sp.tile([128, 24], F32, name="win", tag="win")
                            for r in range(3):
                                nc.vector.max(win[:, 8 * r:8 * r + 8], cand[:])
                                if r < 2:
                                    nc.vector.match_replace(
                                        cand[:], win[:, 8 * r:8 * r + 8],
                                        cand[:], NEG)
                            iu = sp.tile([128, 24], U32, name="iu", tag="iu")
                            nc.vector.tensor_scalar(
                                iu[:], win[:].bitcast(U32), mask_lo[:], None,
                                ALU.bitwise_and)
                            idf = sp.tile([128, 24], F32, name="idf", tag="idf")
                            nc.vector.tensor_copy(idf[:], iu[:])
                            ipt = pp1.tile([24, 128], F32, name="ipt", tag="ipt")
                            nc.tensor.matmul(ipt[:], idf[:], ident[:],
                                             is_transpose=True)
                            nc.vector.tensor_copy(
                                idxT_sb[:, t * 128:(t + 1) * 128], ipt[0:K, :])

                        # ---------- phase 2: v and uT ----------
                        # v[p, :] = x_p @ wbot   (natural layout, bf16 -> DRAM)
                        for pt in range(P // 128):
                            vps = pp1.tile([128, DH], F32, name="vps", tag="vps")
                            if li == 1:
                                nc.tensor.matmul(vps[:],
                                                 Xp1[0:3, pt * 128:(pt + 1) * 128],
                                                 Wb_[0][:], start=True, stop=True)
                            else:
                                for ci in range(NDC):
                                    nc.tensor.matmul(
                                        vps[:],
                                        feat_chunks[ci][:, pt * 128:(pt + 1) * 128],
                                        Wb_[ci][:],
                                        start=(ci == 0), stop=(ci == NDC - 1))
                            vsb = sp.tile([128, DH], BF, name="vsb", tag="vsb")
                            nc.scalar.activation(vsb[:], vps[:], AF.Copy)
                            nc.sync.dma_start(v_dram[pt * 128:(pt + 1) * 128, 0:DH],
                                              vsb[:])

                        # uT[dh, slot] = (x_q @ wdiff + ba).T in slot order
                        uT = []
                        for h in range(NHC):
                            rows = min(128, DH - h * 128)
                            ut = lp.tile([rows, NQ], BF, name=f"uT{h}")
                            for g in range(2):
                                ups = pp1.tile([rows, 512], F32, name="ups",
                                               tag="vps")
                                gg, s0 = divmod(512 * g, nidx)
                                s0 //= 16
                                for ci in range(NDC if li > 1 else 1):
                                    qp = qperm[ci][:, gg, s0:s0 + 32, :]
                                    nc.tensor.matmul(
                                        ups[:], Wd[ci][:, h * 128:h * 128 + rows],
                                        qp, start=(ci == 0), stop=False)
                                nc.tensor.matmul(
                                    ups[:], Ba[:, h * 128:h * 128 + rows],
                                    ones_r[:, 0:512] if li == 1 else ones_bf[:, 0:512],
                                    start=False, stop=True)
                                nc.scalar.activation(
                                    ut[:, g * 512:(g + 1) * 512], ups[:],
                                    AF.Copy)
                            uT.append(ut)

                    # ---------- idx -> DRAM -> wrapped ----------
                    nc.sync.dma_start(idxT_dram[:, :], idxT_sb[:])
                    wrapped = lp.tile([128, K * NGC * NC], I16, name="wrapped")
                    # wrapped16[q, (k*NGC+g)*NC + s] = idxT[k, g*nidx + q*NC + s]
                    vsrc = idxT_dram[:, :].rearrange(
                        "k (g q s) -> q k g s", g=NGC, q=16, s=NC)
                    wdst = wrapped[0:16, :].rearrange(
                        "q (k g s) -> q k g s", k=K, g=NGC)
                    nc.sync.dma_start(wdst, vsrc)
                    for rr in range(1, 8):
                        nc.sync.dma_start(wrapped[16 * rr:16 * (rr + 1), :],
                                          wrapped[0:16, :])

                    # ---------- phase 3: gather + edge MLP + max ----------
                    acc = [fpool.tile([min(128, DO - m * 128), NQ], BF,
                                      name=f"acc{li}_{m}") for m in range(NMC)]
                    with tc.tile_pool(name=f"l{li}p3", bufs=4, space="PSUM") as pp3, \
                         tc.tile_pool(name=f"l{li}g", bufs=2) as gp:
                        for k in range(K):
                            for g in range(NGC):
                                vk = gp.tile([128, cdiv(DHP, 128), nidx], BF,
                                             name="vk", tag="vk")
                                nc.gpsimd.dma_gather(
                                    out_ap=vk[:], in_ap=v_dram[:, :],
                                    idxs_ap=wrapped[:, (k * NGC + g) * NC:
                                                    (k * NGC + g + 1) * NC],
                                    num_idxs=nidx, num_idxs_reg=nidx,
                                    elem_size=DHP, transpose=True,
                                    queue_num=k % 4)
                                h1 = gp.tile([128, NHC, nidx], BF, name="h1",
                                             tag="h1")
                                for h in range(NHC):
                                    rows = min(128, DH - h * 128)
                                    nc.vector.tensor_tensor(
                                        h1[0:rows, h, :], vk[0:rows, h, :],
                                        uT[h][:, g * nidx:(g + 1) * nidx],
                                        op=ALU.add)
                                    nc.vector.tensor_scalar_max(
                                        h1[0:rows, h, :], h1[0:rows, h, :], 0.0)
                                for m in range(NMC):
                                    mrows = min(128, DO - m * 128)
                                    h2ps = pp3.tile([mrows, nidx], F32,
                                                    name="h2ps", tag="h2ps")
                                    for half in range(nidx // 512):
                                        hsl = slice(half * 512, (half + 1) * 512)
                                        for h in range(NHC):
                                            rows = min(128, DH - h * 128)
                                            nc.tensor.matmul(
                                                h2ps[:, hsl],
                                                Wwb[h][:, m * 128:m * 128 + mrows],
                                                h1[0:rows, h, hsl],
                                                start=(h == 0),
                                                stop=(h == NHC - 1))
                                    # relu+bias, unwrap slot->natural order:
                                    # psum col j=s*16+q -> natural col q*64+s
                                    src = h2ps[:].rearrange(
                                        "m (s q) -> m s q", q=16)
                                    if k == 0:
                                        dst = acc[m][:, g * nidx:(g + 1) * nidx] \
                                            .rearrange("m (q s) -> m s q", q=16)
                                        nc.scalar.activation(
                                            dst, src, AF.Relu,
                                            bias=bbt[m][:])
                                    else:
                                        h2sb = gp.tile([mrows, nidx], BF,
                                                       name="h2sb", tag="h2sb")
                                        dst2 = h2sb[:].rearrange(
                                            "m (q s) -> m s q", q=16)
                                        nc.scalar.activation(
                                            dst2, src, AF.Relu,
                                            bias=bbt[m][:])
                                        nc.vector.tensor_tensor(
                                            acc[m][:, g * nidx:(g + 1) * nidx],
                                            h2sb[:],
                                            acc[m][:, g * nidx:(g + 1) * nidx],
                                            op=ALU.max)

                    acc_tiles[li] = acc
                    for m in range(NMC):
                        mrows = min(128, DO - m * 128)
                        nc.sync.dma_start(
                            xown_dram[li - 1][m * 128:m * 128 + mrows, :],
                            acc[m][:])

                    if li < 4:
                        nc.gpsimd.collective_compute(
                            "AllGather", ALU.bypass, replica_groups=groups,
                            ins=[xown_dram[li - 1].opt()],
                            outs=[ag_out[li - 1].opt()])

                if li < 4:
                    DO_ = DO
                    with tc.tile_pool(name=f"x{li}ps", bufs=1, space="PSUM") as xpp, \
                         tc.tile_pool(name=f"x{li}tmp", bufs=2) as xtmp:
                        feat_chunks = []
                        for m in range(cdiv(DO_, 128)):
                            mrows = min(128, DO_ - m * 128)
                            xt = fpool.tile([mrows, P], BF, name=f"xTn{li}_{m}",
                                            tag=f"xTn{li}_{m}")
                            nc.sync.dma_start(
                                xt[:, 0:NQ],
                                ag_out[li - 1][m * 128:m * 128 + mrows, :])
                            nc.sync.dma_start(
                                xt[:, NQ:P],
                                ag_out[li - 1][DO_ + m * 128:DO_ + m * 128 + mrows, :])
                            feat_chunks.append(xt)
                        sps = xpp.tile([1, P], F32, name="sps")
                        for m, xt in enumerate(feat_chunks):
                            mrows = xt.shape[0]
                            sq = xtmp.tile([mrows, P], BF, name="sq", tag="sq")
                            nc.vector.tensor_tensor(sq[:], xt[:], xt[:],
                                                    op=ALU.mult)
                            for nb in range(P // 512):
                                nc.tensor.matmul(
                                    sps[:, nb * 512:(nb + 1) * 512],
                                    onescol_bf[0:mrows, :],
                                    sq[:, nb * 512:(nb + 1) * 512],
                                    start=(m == 0),
                                    stop=(m == len(feat_chunks) - 1))
                        nsq_row = fpool.tile([1, P], BF, name=f"nsqr{li}",
                                             tag=f"nsqr{li}")
                        nc.scalar.activation(nsq_row[:], sps[:], AF.Copy,
                                             scale=-0.5)

            # ---------------- FC head ----------------
            feats = []
            for li in range(1, 5):
                feats.extend(acc_tiles[li])
            fchunks = [64, 128, 128, 128, 128, 128, 128, 128]
            with tc.tile_pool(name="fc", bufs=1) as fp, \
                 tc.tile_pool(name="fcw", bufs=1) as fw, \
                 tc.tile_pool(name="fcps", bufs=4, space="PSUM") as fpp:
                Wf1 = load_bf(fw, "fw1", (960, 512), row_chunks=fchunks)
                Wf2 = load_bf(fw, "fw2", (512, 256))
                Wf3 = load_bf(fw, "fw3", (256, 1))
                def load_bias(name, nout):
                    ts = []
                    for m in range(cdiv(nout, 128)):
                        mrows = min(128, nout - m * 128)
                        bt = fw.tile([mrows, 1], F32, name=f"{name}t{m}")
                        nc.sync.dma_start(
                            bt[:], wparams[name][m * 128:m * 128 + mrows, :])
                        ts.append(bt)
                    return ts

                fb1t = load_bias("fb1", 512)
                fb2t = load_bias("fb2", 256)
                fb3t = load_bias("fb3", 1)

                def fc_layer(rhs, wtiles, btile, nout, act, outdt):
                    outs = []
                    for m in range(cdiv(nout, 128)):
                        mrows = min(128, nout - m * 128)
                        ot = fp.tile([mrows, NQ], outdt, name=f"fco{nout}_{m}")
                        for g in range(2):
                            ps = fpp.tile([mrows, 512], F32, name="fps",
                                          tag="fcps")
                            for ci, rc in enumerate(rhs):
                                nc.tensor.matmul(
                                    ps[:], wtiles[ci][:, m * 128:m * 128 + mrows],
                                    rc[:, g * 512:(g + 1) * 512],
                                    start=(ci == 0), stop=(ci == len(rhs) - 1))
                            nc.scalar.activation(
                                ot[:, g * 512:(g + 1) * 512], ps[:], act,
                                bias=btile[m][:])
                        outs.append(ot)
                    return outs

                h1fc = fc_layer(feats, Wf1, fb1t, 512, AF.Relu, BF)
                h2fc = fc_layer(h1fc, Wf2, fb2t, 256, AF.Relu, BF)
                yt = fc_layer(h2fc, Wf3, fb3t, 1, AF.Sigmoid, F32)
                nc.sync.dma_start(y_out[:, :], yt[0][:])

    nc.compile()
    return nc


def kernel(**inputs):
    x = np.asarray(inputs["x"], np.float32)          # [8192, 3]
    if "nc" not in _CACHED:
        _CACHED["nc"] = _build()
    nc = _CACHED["nc"]

    base = {}
    for li in range(1, 5):
        wa = np.asarray(inputs[f"w{li}a"], np.float32)
        D = wa.shape[0] // 2
        base[f"wdiff{li}"] = np.ascontiguousarray(wa[:D] - wa[D:])
        base[f"wbot{li}"] = np.ascontiguousarray(wa[D:])
        base[f"ba{li}"] = np.asarray(inputs[f"b{li}a"], np.float32)[None, :]
        base[f"wb{li}"] = np.asarray(inputs[f"w{li}b"], np.float32)
        base[f"bb{li}"] = np.asarray(inputs[f"b{li}b"], np.float32)[:, None]
    base["fw1"] = np.asarray(inputs["fw1"], np.float32)
    base["fw2"] = np.asarray(inputs["fw2"], np.float32)
    base["fw3"] = np.asarray(inputs["fw3"], np.float32)
    base["fb1"] = np.asarray(inputs["fb1"], np.float32)[:, None]
    base["fb2"] = np.asarray(inputs["fb2"], np.float32)[:, None]
    base["fb3"] = np.asarray(inputs["fb3"], np.float32)[None, :]
    base["iota"] = np.arange(P, dtype=np.uint32).view(np.float32)[None, :]

    in_maps = []
    for c in range(N_CORES):
        cloud = c // 2
        xc = x[cloud * P:(cloud + 1) * P]
        m = dict(base)
        m["xT"] = np.ascontiguousarray(xc.T)
        m["nsq"] = (-0.5 * (xc * xc).sum(1))[None, :].astype(np.float32)
        in_maps.append(m)

    res = run_bass_kernel_spmd(nc, in_maps, core_ids=list(range(N_CORES)))
    out = np.empty((B * P, 1), np.float32)
    for c in range(N_CORES):
        cloud, half = c // 2, c % 2
        out[cloud * P + half * NQ: cloud * P + (half + 1) * NQ, 0] = \
            res.results[c]["y"][0]
    return out


# revision 39
# speedup vs baseline: 1.3304x; 1.2693x over previous
"""DGCNN (4x EdgeConv + FC head) Bass kernel for 8 trn2 NeuronCores — v2.

Sharding: cloud b -> cores {2b, 2b+1}; each core owns NQ=1024 query points
(q0 = (partition_id % 2) * 1024). Full-cloud features exchanged within each
pair via bf16 AllGather after layers 1-3.

v2 design vs v1:
- dist matmul: rank-1 fold of -0.5|x|^2 (and bias) into the matmul; bf16
  features for layers 2-4 (f32r for layer 1 coords).
- topk: index bits embedded into score mantissa (score&~0x7FF)|iota, then
  chunked max8 (8x256) + top-24-of-64 rounds; indices recovered by masking.
  No MaxIndex scans at all.
- neighbor features v stored bf16 in DRAM; gathered with transpose-mode
  dma_gather directly into [DH, edge] layout (no per-edge PE transposes).
- h1 = relu(uT + vT) on DVE (bf16 2x/4x); h2 = wb.T @ h1 (bf16 matmuls);
- per-k relu+bias on ACT (psum->sbuf bf16, unwrapping the gather slot
  permutation via a strided write AP); max over k on DVE in bf16.
- FC head runs from SBUF-resident features, biases applied as ACT bias.
"""
import numpy as np

import concourse.bass as bass
import concourse.bacc as bacc
import concourse.mybir as mybir
import concourse.tile as tile
from concourse.bass_utils import run_bass_kernel_spmd
from concourse.masks import make_identity

B, P, K = 4, 2048, 20
NQ = 1024
N_CORES = 8
NEG = -3.0e38
LDIMS = [(3, 64, 64), (64, 128, 128), (128, 256, 256), (256, 512, 512)]
F32 = mybir.dt.float32
F32R = mybir.dt.float32r
BF = mybir.dt.bfloat16
U32 = mybir.dt.uint32
I16 = mybir.dt.int16
AF = mybir.ActivationFunctionType
ALU = mybir.AluOpType
NIDX = 512           # indices per gather call (per-queue SWDGE ring = 1024 descs)
NCH = 8              # score chunks for phase-A max8
CHW = P // NCH       # 256

_CACHED = {}


def cdiv(a, b):
    return (a + b - 1) // b


def _build(nidx=NIDX):
    nc = bacc.Bacc("TRN2", target_bir_lowering=False, debug=False,
                   num_devices=N_CORES, num_swdge_queues=4)

    xT_in = nc.declare_dram_parameter("xT", [3, P], F32, isOutput=False)
    nsq_in = nc.declare_dram_parameter("nsq", [1, P], F32, isOutput=False)
    iota_in = nc.declare_dram_parameter("iota", [1, P], F32, isOutput=False)
    wparams = {}
    for li, (D, DH, DO) in enumerate(LDIMS, start=1):
        wparams[f"wdiff{li}"] = nc.declare_dram_parameter(f"wdiff{li}", [D, DH], F32, isOutput=False)
        wparams[f"wbot{li}"] = nc.declare_dram_parameter(f"wbot{li}", [D, DH], F32, isOutput=False)
        wparams[f"ba{li}"] = nc.declare_dram_parameter(f"ba{li}", [1, DH], F32, isOutput=False)
        wparams[f"wb{li}"] = nc.declare_dram_parameter(f"wb{li}", [DH, DO], F32, isOutput=False)
        wparams[f"bb{li}"] = nc.declare_dram_parameter(f"bb{li}", [DO, 1], F32, isOutput=False)
    wparams["fw1"] = nc.declare_dram_parameter("fw1", [960, 512], F32, isOutput=False)
    wparams["fb1"] = nc.declare_dram_parameter("fb1", [512, 1], F32, isOutput=False)
    wparams["fw2"] = nc.declare_dram_parameter("fw2", [512, 256], F32, isOutput=False)
    wparams["fb2"] = nc.declare_dram_parameter("fb2", [256, 1], F32, isOutput=False)
    wparams["fw3"] = nc.declare_dram_parameter("fw3", [256, 1], F32, isOutput=False)
    wparams["fb3"] = nc.declare_dram_parameter("fb3", [1, 1], F32, isOutput=False)
    y_out = nc.declare_dram_parameter("y", [1, NQ], F32, isOutput=True)

    groups = [[2 * b, 2 * b + 1] for b in range(N_CORES // 2)]
    NC = nidx // 16      # idx columns per gather call in wrapped layout
    NGC = NQ // nidx     # gather calls per k (1 when nidx=1024)

    with tile.TileContext(nc) as tc:
        with tc.tile_pool(name="const", bufs=1) as cpool, \
             tc.tile_pool(name="feat", bufs=1) as fpool, \
             tc.tile_pool(name="dram", bufs=1, space="DRAM") as dram:

            ident = cpool.tile([128, 128], F32)
            make_identity(nc, ident[:])
            ones_bf = cpool.tile([1, P], BF)
            nc.vector.memset(ones_bf[:], 1.0)
            ones_f = cpool.tile([1, NQ], F32)
            nc.vector.memset(ones_f[:], 1.0)
            ones_r = cpool.tile([1, NQ], F32R)
            nc.vector.tensor_copy(ones_r[:], ones_f[:])
            onescol_bf = cpool.tile([128, 1], BF)
            nc.vector.memset(onescol_bf[:], 1.0)
            iota_bc = cpool.tile([128, P], F32)
            nc.sync.dma_start(iota_bc[:], iota_in[0:1, :].to_broadcast([128, P]))
            mask_hi = cpool.tile([128, 1], U32)
            nc.vector.memset(mask_hi[:], 0xFFFFF800)
            mask_lo = cpool.tile([128, 1], U32)
            nc.vector.memset(mask_lo[:], 0x7FF)

            q0 = nc.vector.partition_id()
            q0 = nc.vector.scalar_reg_alu(ALU.mod, q0, 2)
            q0 = nc.vector.scalar_reg_alu(ALU.mult, q0, NQ)

            def load_bf(pool, name, shape, row_chunks=None, tagpfx=None):
                """DRAM fp32 -> SBUF bf16 tiles chunked along rows."""
                src = wparams[name]
                if row_chunks is None:
                    row_chunks = []
                    r = shape[0]
                    while r > 0:
                        row_chunks.append(min(128, r))
                        r -= 128
                tiles, c0 = [], 0
                tagpfx = tagpfx or name
                for rows in row_chunks:
                    t32 = pool.tile([rows, shape[1]], F32, name=f"{name}_f{c0}",
                                    tag="wstage", bufs=2)
                    nc.sync.dma_start(t32[:], src[c0:c0 + rows, :])
                    tb = pool.tile([rows, shape[1]], BF, name=f"{name}_b{c0}",
                                   tag=f"{tagpfx}_b{c0}")
                    nc.vector.tensor_copy(tb[:], t32[:])
                    tiles.append(tb)
                    c0 += rows
                return tiles

            # ---- layer-1 input staging (f32r coords + nsq) ----
            Xp1 = fpool.tile([4, P], F32R, name="Xp1")
            Qp1 = fpool.tile([4, NQ], F32R, name="Qp1")
            with tc.tile_pool(name="stage1", bufs=1) as st1:
                # assemble [x; nsq] and [x_q; ones] in fp32 via DMA (which has
                # no partition-alignment limits), then round to f32r in one
                # aligned DVE copy each.
                x4f = st1.tile([4, P], F32, name="x4f")
                nc.sync.dma_start(x4f[0:3, :], xT_in[:, :])
                nc.sync.dma_start(x4f[3:4, :], nsq_in[0:1, :])
                nc.vector.tensor_copy(Xp1[:], x4f[:])
                q4f = st1.tile([4, NQ], F32, name="q4f")
                nc.vector.tensor_copy(q4f[0:3, :], x4f[0:3, bass.ds(q0, NQ)])
                nc.sync.dma_start(q4f[3:4, :], ones_f[:])
                nc.vector.tensor_copy(Qp1[:], q4f[:])

            # persistent per-layer outputs (own half, natural order, bf16)
            acc_tiles = {}      # li -> list of [mrows, NQ] bf16 tiles
            xown_dram = [dram.tile([do, NQ], BF, name=f"xown{li}")
                         for li, (_, _, do) in enumerate(LDIMS, start=1)]
            ag_out = [dram.tile([2 * do, NQ], BF, name=f"agout{li}")
                      for li, (_, _, do) in enumerate(LDIMS[:3], start=1)]

            # feature chunks for the NEXT layer (full cloud, bf16) and nsq row
            feat_chunks = None   # list of [rows<=128, P] bf16 tiles
            nsq_row = None       # [1, P] bf16

            for li, (D, DH, DO) in enumerate(LDIMS, start=1):
                NDC = cdiv(D, 128)     # feature chunks (layers 2-4)
                NHC = cdiv(DH, 128)
                NMC = cdiv(DO, 128)
                DHP = max(DH, 128)     # padded row length of v in DRAM
                v_dram = dram.tile([P, DHP], BF, name=f"vdram{li}")
                idxT_dram = dram.tile([K, NQ], I16, name=f"idxTd{li}")

                with tc.tile_pool(name=f"l{li}w", bufs=1) as lw, \
                     tc.tile_pool(name=f"l{li}f", bufs=1) as lp, \
                     tc.tile_pool(name=f"l{li}s", bufs=2) as sp:

                    if li == 1:
                        Wd = [cpool.tile([3, DH], F32R, name="wd1r")]
                        Wb_ = [cpool.tile([3, DH], F32R, name="wb1r")]
                        Ba = cpool.tile([1, DH], F32R, name="ba1r")
                        for nm, dst in (("wdiff1", Wd[0]), ("wbot1", Wb_[0]),
                                        ("ba1", Ba)):
                            t32 = lw.tile(list(dst.shape), F32, name=f"{nm}_f",
                                          tag="wstage1", bufs=2)
                            nc.sync.dma_start(t32[:], wparams[nm][:, :])
                            nc.vector.tensor_copy(dst[:], t32[:])
                    else:
                        Wd = load_bf(lw, f"wdiff{li}", (D, DH))
                        Wb_ = load_bf(lw, f"wbot{li}", (D, DH))
                        Ba = load_bf(lw, f"ba{li}", (1, DH))[0]
                    Wwb = load_bf(lw, f"wb{li}", (DH, DO))
                    bbt = []
                    for m in range(NMC):
                        mrows = min(128, DO - m * 128)
                        bt = lw.tile([mrows, 1], F32, name=f"bb{li}t{m}",
                                     tag=f"bb{li}t{m}")
                        nc.sync.dma_start(
                            bt[:], wparams[f"bb{li}"][m * 128:m * 128 + mrows, :])
                        bbt.append(bt)

                    # query columns: own-half features are the previous
                    # layer's acc tiles (pre-collective, so uT/dist-lhsT
                    # work can overlap the AllGather)
                    if li == 1:
                        qfeat = [Qp1[0:3, :]]
                    else:
                        qfeat = [a[:] for a in acc_tiles[li - 1]]
                    # slot-space view per gather group of `nidx` queries:
                    # slot j = s*16+q -> query g*nidx + q*NC + s
                    qperm = [ap.rearrange("d (g q s) -> d g s q",
                                          g=NGC, q=16, s=NC)
                             for ap in qfeat]
                    # idx scratch
                    idxT_sb = lp.tile([K, NQ], I16, name="idxT")

                    # ---------- phase 1: dist + topk ----------
                    with tc.tile_pool(name=f"l{li}p1", bufs=2, space="PSUM") as pp1:

                        # uT[dh, slot] = (x_q @ wdiff + ba).T in slot order
                        uT = []
                        for h in range(NHC):
                            rows = min(128, DH - h * 128)
                            ut = lp.tile([rows, NQ], BF, name=f"uT{h}")
                            for g in range(2):
                                ups = pp1.tile([rows, 512], F32, name="ups",
                                               tag="vps")
                                gg, s0 = divmod(512 * g, nidx)
                                s0 //= 16
                                for ci in range(NDC if li > 1 else 1):
                                    qp = qperm[ci][:, gg, s0:s0 + 32, :]
                                    nc.tensor.matmul(
                                        ups[:], Wd[ci][:, h * 128:h * 128 + rows],
                                        qp, start=(ci == 0), stop=False)
                                nc.tensor.matmul(
                                    ups[:], Ba[:, h * 128:h * 128 + rows],
                                    ones_r[:, 0:512] if li == 1 else ones_bf[:, 0:512],
                                    start=False, stop=True)
                                nc.scalar.activation(
                                    ut[:, g * 512:(g + 1) * 512], ups[:],
                                    AF.Copy)
                            uT.append(ut)

                        for t in range(NQ // 128):
                            score = sp.tile([128, P], F32, name="score", tag="score")
                            for hb in range(2):
                                dps = pp1.tile([128, 1024], F32, name="dps", tag="dps")
                                for nb in range(2):
                                    psl = slice(nb * 512, (nb + 1) * 512)
                                    sl = slice(hb * 1024 + nb * 512,
                                               hb * 1024 + (nb + 1) * 512)
                                    if li == 1:
                                        nc.tensor.matmul(
                                            dps[:, psl],
                                            Qp1[:, t * 128:(t + 1) * 128],
                                            Xp1[:, sl], start=True, stop=True)
                                    else:
                                        for ci in range(NDC):
                                            nc.tensor.matmul(
                                                dps[:, psl],
                                                qfeat[ci][:, t * 128:(t + 1) * 128],
                                                feat_chunks[ci][:, sl],
                                                start=(ci == 0), stop=False)
                                        nc.tensor.matmul(
                                            dps[:, psl],
                                            ones_bf[:, t * 128:(t + 1) * 128],
                                            nsq_row[:, sl],
                                            start=False, stop=True)
                                nc.scalar.activation(
                                    score[:, hb * 1024:(hb + 1) * 1024], dps[:],
                                    AF.Copy)
                            # embed candidate index into low mantissa bits
                            nc.vector.scalar_tensor_tensor(
                                score[:].bitcast(U32), score[:].bitcast(U32),
                                mask_hi[:], iota_bc[:].bitcast(U32),
                                ALU.bitwise_and, ALU.bitwise_or)
                            cand = sp.tile([128, 8 * NCH], F32, name="cand",
                                           tag="cand")
                            for c in range(NCH):
                                nc.vector.max(cand[:, 8 * c:8 * c + 8],
                                              score[:, CHW * c:CHW * (c + 1)])
                            win = sp.tile([128, 24], F32, name="win", tag="win")
                            for r in range(3):
                                nc.vector.max(win[:, 8 * r:8 * r + 8], cand[:])
                                if r < 2:
                                    nc.vector.match_replace(
                                        cand[:], win[:, 8 * r:8 * r + 8],
                                        cand[:], NEG)
                            iu = sp.tile([128, 24], U32, name="iu", tag="iu")
                            nc.vector.tensor_scalar(
                                iu[:], win[:].bitcast(U32), mask_lo[:], None,
                                ALU.bitwise_and)
                            idf = sp.tile([128, 24], F32, name="idf", tag="idf")
                            nc.vector.tensor_copy(idf[:], iu[:])
                            ipt = pp1.tile([24, 128], F32, name="ipt", tag="ipt")
                            nc.tensor.matmul(ipt[:], idf[:], ident[:],
                                             is_transpose=True)
                            nc.vector.tensor_copy(
                                idxT_sb[:, t * 128:(t + 1) * 128], ipt[0:K, :])

                        # ---------- phase 2: v and uT ----------
                        # v[p, :] = x_p @ wbot   (natural layout, bf16 -> DRAM)
                        for pt in range(P // 128):
                            vps = pp1.tile([128, DH], F32, name="vps", tag="vps")
                            if li == 1:
                                nc.tensor.matmul(vps[:],
                                                 Xp1[0:3, pt * 128:(pt + 1) * 128],
                                                 Wb_[0][:], start=True, stop=True)
                            else:
                                for ci in range(NDC):
                                    nc.tensor.matmul(
                                        vps[:],
                                        feat_chunks[ci][:, pt * 128:(pt + 1) * 128],
                                        Wb_[ci][:],
                                        start=(ci == 0), stop=(ci == NDC - 1))
                            vsb = sp.tile([128, DH], BF, name="vsb", tag="vsb")
                            nc.scalar.activation(vsb[:], vps[:], AF.Copy)
                            nc.sync.dma_start(v_dram[pt * 128:(pt + 1) * 128, 0:DH],
                                              vsb[:])


                    # ---------- idx -> DRAM -> wrapped ----------
                    nc.sync.dma_start(idxT_dram[:, :], idxT_sb[:])
                    wrapped = lp.tile([128, K * NGC * NC], I16, name="wrapped")
                    # wrapped16[q, (k*NGC+g)*NC + s] = idxT[k, g*nidx + q*NC + s]
                    vsrc = idxT_dram[:, :].rearrange(
                        "k (g q s) -> q k g s", g=NGC, q=16, s=NC)
                    wdst = wrapped[0:16, :].rearrange(
                        "q (k g s) -> q k g s", k=K, g=NGC)
                    nc.sync.dma_start(wdst, vsrc)
                    for rr in range(1, 8):
                        nc.sync.dma_start(wrapped[16 * rr:16 * (rr + 1), :],
                                          wrapped[0:16, :])

                    # ---------- phase 3: gather + edge MLP + max ----------
                    acc = [fpool.tile([min(128, DO - m * 128), NQ], BF,
                                      name=f"acc{li}_{m}") for m in range(NMC)]
                    with tc.tile_pool(name=f"l{li}p3", bufs=4, space="PSUM") as pp3, \
                         tc.tile_pool(name=f"l{li}g", bufs=2) as gp:
                        for k in range(K):
                            for g in range(NGC):
                                vk = gp.tile([128, cdiv(DHP, 128), nidx], BF,
                                             name="vk", tag="vk")
                                nc.gpsimd.dma_gather(
                                    out_ap=vk[:], in_ap=v_dram[:, :],
                                    idxs_ap=wrapped[:, (k * NGC + g) * NC:
                                                    (k * NGC + g + 1) * NC],
                                    num_idxs=nidx, num_idxs_reg=nidx,
                                    elem_size=DHP, transpose=True,
                                    queue_num=k % 4)
                                h1 = gp.tile([128, NHC, nidx], BF, name="h1",
                                             tag="h1")
                                for h in range(NHC):
                                    rows = min(128, DH - h * 128)
                                    nc.vector.tensor_tensor(
                                        h1[0:rows, h, :], vk[0:rows, h, :],
                                        uT[h][:, g * nidx:(g + 1) * nidx],
                                        op=ALU.add)
                                    nc.vector.tensor_scalar_max(
                                        h1[0:rows, h, :], h1[0:rows, h, :], 0.0)
                                for m in range(NMC):
                                    mrows = min(128, DO - m * 128)
                                    h2ps = pp3.tile([mrows, nidx], F32,
                                                    name="h2ps", tag="h2ps")
                                    for half in range(nidx // 512):
                                        hsl = slice(half * 512, (half + 1) * 512)
                                        for h in range(NHC):
                                            rows = min(128, DH - h * 128)
                                            nc.tensor.matmul(
                                                h2ps[:, hsl],
                                                Wwb[h][:, m * 128:m * 128 + mrows],
                                                h1[0:rows, h, hsl],
                                                start=(h == 0),
                                                stop=(h == NHC - 1))
                                    # relu+bias, unwrap slot->natural order:
                                    # psum col j=s*16+q -> natural col q*64+s
                                    src = h2ps[:].rearrange(
                                        "m (s q) -> m s q", q=16)
                                    if k == 0:
                                        dst = acc[m][:, g * nidx:(g + 1) * nidx] \
                                            .rearrange("m (q s) -> m s q", q=16)
                                        nc.scalar.activation(
                                            dst, src, AF.Relu,
                                            bias=bbt[m][:])
                                    else:
                                        h2sb = gp.tile([mrows, nidx], BF,
                                                       name="h2sb", tag="h2sb")
                                        dst2 = h2sb[:].rearrange(
                                            "m (q s) -> m s q", q=16)
                                        nc.scalar.activation(
                                            dst2, src, AF.Relu,
                                            bias=bbt[m][:])
                                        nc.vector.tensor_tensor(
                                            acc[m][:, g * nidx:(g + 1) * nidx],
                                            h2sb[:],
                                            acc[m][:, g * nidx:(g + 1) * nidx],
                                            op=ALU.max)

                    acc_tiles[li] = acc
                    for m in range(NMC):
                        mrows = min(128, DO - m * 128)
                        nc.sync.dma_start(
                            xown_dram[li - 1][m * 128:m * 128 + mrows, :],
                            acc[m][:])

                    if li < 4:
                        nc.gpsimd.collective_compute(
                            "AllGather", ALU.bypass, replica_groups=groups,
                            ins=[xown_dram[li - 1].opt()],
                            outs=[ag_out[li - 1].opt()])

                if li < 4:
                    DO_ = DO
                    with tc.tile_pool(name=f"x{li}ps", bufs=1, space="PSUM") as xpp, \
                         tc.tile_pool(name=f"x{li}tmp", bufs=2) as xtmp:
                        feat_chunks = []
                        for m in range(cdiv(DO_, 128)):
                            mrows = min(128, DO_ - m * 128)
                            xt = fpool.tile([mrows, P], BF, name=f"xTn{li}_{m}",
                                            tag=f"xTn{li}_{m}")
                            nc.sync.dma_start(
                                xt[:, 0:NQ],
                                ag_out[li - 1][m * 128:m * 128 + mrows, :])
                            nc.sync.dma_start(
                                xt[:, NQ:P],
                                ag_out[li - 1][DO_ + m * 128:DO_ + m * 128 + mrows, :])
                            feat_chunks.append(xt)
                        sps = xpp.tile([1, P], F32, name="sps")
                        for m, xt in enumerate(feat_chunks):
                            mrows = xt.shape[0]
                            sq = xtmp.tile([mrows, P], BF, name="sq", tag="sq")
                            nc.vector.tensor_tensor(sq[:], xt[:], xt[:],
                                                    op=ALU.mult)
                            for nb in range(P // 512):
                                nc.tensor.matmul(
                                    sps[:, nb * 512:(nb + 1) * 512],
                                    onescol_bf[0:mrows, :],
                                    sq[:, nb * 512:(nb + 1) * 512],
                                    start=(m == 0),
                                    stop=(m == len(feat_chunks) - 1))
                        nsq_row = fpool.tile([1, P], BF, name=f"nsqr{li}",
                                             tag=f"nsqr{li}")
                        nc.scalar.activation(nsq_row[:], sps[:], AF.Copy,
                                             scale=-0.5)

            # ---------------- FC head ----------------
            feats = []
            for li in range(1, 5):
                feats.extend(acc_tiles[li])
            fchunks = [64, 128, 128, 128, 128, 128, 128, 128]
            with tc.tile_pool(name="fc", bufs=1) as fp, \
                 tc.tile_pool(name="fcw", bufs=1) as fw, \
                 tc.tile_pool(name="fcps", bufs=4, space="PSUM") as fpp:
                Wf1 = load_bf(fw, "fw1", (960, 512), row_chunks=fchunks)
                Wf2 = load_bf(fw, "fw2", (512, 256))
                Wf3 = load_bf(fw, "fw3", (256, 1))
                def load_bias(name, nout):
                    ts = []
                    for m in range(cdiv(nout, 128)):
                        mrows = min(128, nout - m * 128)
                        bt = fw.tile([mrows, 1], F32, name=f"{name}t{m}")
                        nc.sync.dma_start(
                            bt[:], wparams[name][m * 128:m * 128 + mrows, :])
                        ts.append(bt)
                    return ts

                fb1t = load_bias("fb1", 512)
                fb2t = load_bias("fb2", 256)
                fb3t = load_bias("fb3", 1)

                def fc_layer(rhs, wtiles, btile, nout, act, outdt):
                    outs = []
                    for m in range(cdiv(nout, 128)):
                        mrows = min(128, nout - m * 128)
                        ot = fp.tile([mrows, NQ], outdt, name=f"fco{nout}_{m}")
                        for g in range(2):
                            ps = fpp.tile([mrows, 512], F32, name="fps",
                                          tag="fcps")
                            for ci, rc in enumerate(rhs):
                                nc.tensor.matmul(
                                    ps[:], wtiles[ci][:, m * 128:m * 128 + mrows],
                                    rc[:, g * 512:(g + 1) * 512],
                                    start=(ci == 0), stop=(ci == len(rhs) - 1))
                            nc.scalar.activation(
                                ot[:, g * 512:(g + 1) * 512], ps[:], act,
                                bias=btile[m][:])
                        outs.append(ot)
                    return outs

                h1fc = fc_layer(feats, Wf1, fb1t, 512, AF.Relu, BF)
                h2fc = fc_layer(h1fc, Wf2, fb2t, 256, AF.Relu, BF)
                yt = fc_layer(h2fc, Wf3, fb3t, 1, AF.Sigmoid, F32)
                nc.sync.dma_start(y_out[:, :], yt[0][:])

    nc.compile()
    return nc


def kernel(**inputs):
    x = np.asarray(inputs["x"], np.float32)          # [8192, 3]
    if "nc" not in _CACHED:
        _CACHED["nc"] = _build()
    nc = _CACHED["nc"]

    base = {}
    for li in range(1, 5):
        wa = np.asarray(inputs[f"w{li}a"], np.float32)
        D = wa.shape[0] // 2
        base[f"wdiff{li}"] = np.ascontiguousarray(wa[:D] - wa[D:])
        base[f"wbot{li}"] = np.ascontiguousarray(wa[D:])
        base[f"ba{li}"] = np.asarray(inputs[f"b{li}a"], np.float32)[None, :]
        base[f"wb{li}"] = np.asarray(inputs[f"w{li}b"], np.float32)
        base[f"bb{li}"] = np.asarray(inputs[f"b{li}b"], np.float32)[:, None]
    base["fw1"] = np.asarray(inputs["fw1"], np.float32)
    base["fw2"] = np.asarray(inputs["fw2"], np.float32)
    base["fw3"] = np.asarray(inputs["fw3"], np.float32)
    base["fb1"] = np.asarray(inputs["fb1"], np.float32)[:, None]
    base["fb2"] = np.asarray(inputs["fb2"], np.float32)[:, None]
    base["fb3"] = np.asarray(inputs["fb3"], np.float32)[None, :]
    base["iota"] = np.arange(P, dtype=np.uint32).view(np.float32)[None, :]

    in_maps = []
    for c in range(N_CORES):
        cloud = c // 2
        xc = x[cloud * P:(cloud + 1) * P]
        m = dict(base)
        m["xT"] = np.ascontiguousarray(xc.T)
        m["nsq"] = (-0.5 * (xc * xc).sum(1))[None, :].astype(np.float32)
        in_maps.append(m)

    res = run_bass_kernel_spmd(nc, in_maps, core_ids=list(range(N_CORES)))
    out = np.empty((B * P, 1), np.float32)
    for c in range(N_CORES):
        cloud, half = c // 2, c % 2
        out[cloud * P + half * NQ: cloud * P + (half + 1) * NQ, 0] = \
            res.results[c]["y"][0]
    return out
